# revision 5
# baseline (speedup 1.0000x reference)
"""GRU (Flax GRUCell scanned over time) on 8 Trainium2 NeuronCores.

Problem: x:[T,B,D]=[512,64,512], h0:[B,H], Wi:[D,3H], Wh:[H,3H], bi:[3H], bhn:[H]
  gi = x_t @ Wi + bi ; gh = h @ Wh ; gates (r,z,n); h' = (1-z)*n + z*h
  returns ys:[T,B,H] (the h trajectory).

Strategy (per core, data-parallel over batch, B_local=8):
  Everything on-chip lives in "T-layout": hidden dim on SBUF partitions,
  batch on the free dim, so elementwise work uses all 128 lanes.
  - Phase 1 (parallel over time): giT = Wi.T @ xT for all (t, b) as one big
    matmul (Wi tiles stationary, bf16), fp32 PSUM, spilled to HBM in a
    recurrence-friendly layout.  bi folded in via a per-partition scalar add.
  - Phase 2 (sequential, 512 steps): ghT = Wh.T @ hT with Wh tiles
    stationary (bf16, FWL), hT streaming (N=B_local).  Gates via ACT
    sigmoid/tanh + DVE ops on [128, *] tiles.  h state stays fp32.
  Output is written transposed; the host reassembles ys.
"""

import warnings

warnings.filterwarnings("ignore")

import numpy as np
import ml_dtypes

import concourse.bacc as bacc
import concourse.tile as tile
from concourse import mybir, bass_utils

B, D, H = 64, 512, 512
NCORES = 8
BL = B // NCORES  # batch per core
KD = D // 128  # input-dim k-chunks
KH = H // 128  # hidden-dim k-chunks
M3 = 3 * H // 128  # 3H m-tiles
RT = 8  # recurrence steps per DMA ring chunk
BF16 = mybir.dt.bfloat16
F32 = mybir.dt.float32
NPBF16 = ml_dtypes.bfloat16

_cache: dict = {}


def _build(T: int, use_bi: bool, use_bhn: bool):
    TB = T * BL
    NT = TB // 512  # phase-1 N-chunks of 512
    assert TB % 512 == 0 and T % RT == 0
    nc = bacc.Bacc("TRN2", target_bir_lowering=False, debug=False, num_devices=NCORES)

    xt_d = nc.dram_tensor("xt", [128, KD * TB], BF16, kind="ExternalInput").ap()
    wi_d = nc.dram_tensor("wi", [128, M3 * KD * 128], BF16, kind="ExternalInput").ap()
    wh_d = nc.dram_tensor("wh", [128, M3 * KH * 128], BF16, kind="ExternalInput").ap()
    h0_d = nc.dram_tensor("h0t", [128, KH * BL], F32, kind="ExternalInput").ap()
    bi_d = (
        nc.dram_tensor("bi_t", [128, M3], F32, kind="ExternalInput").ap()
        if use_bi
        else None
    )
    bhn_d = (
        nc.dram_tensor("bhn_t", [128, KH], F32, kind="ExternalInput").ap()
        if use_bhn
        else None
    )
    gi_d = nc.dram_tensor("gi", [128, M3 * TB], F32, kind="Internal").ap()
    ys_d = nc.dram_tensor("yst", [128, KH * TB], F32, kind="ExternalOutput").ap()

    gi_v = gi_d.rearrange("p (m t j) -> p m t j", m=M3, j=BL)
    ys_v = ys_d.rearrange("p (k t j) -> p k t j", k=KH, j=BL)

    with tile.TileContext(nc) as tc:
        with (
            tc.tile_pool(name="const", bufs=1) as const,
            tc.tile_pool(name="xin", bufs=1) as xin,
            tc.tile_pool(name="p1", bufs=3, space="PSUM") as p1,
            tc.tile_pool(name="stage", bufs=3) as stagep,
            tc.tile_pool(name="girp", bufs=2) as girp,
            tc.tile_pool(name="orp", bufs=3) as orp,
            tc.tile_pool(name="rzp", bufs=2, space="PSUM") as rzp,
            tc.tile_pool(name="np_", bufs=2, space="PSUM") as npp,
            tc.tile_pool(name="hbp", bufs=2) as hbp,
            tc.tile_pool(name="ew", bufs=2) as ew,
        ):
            # ---- load constants ----
            wi_sb = const.tile([128, M3 * KD * 128], BF16)
            nc.sync.dma_start(wi_sb[:], wi_d[:])
            wh_sb = const.tile([128, M3 * KH * 128], BF16)
            nc.sync.dma_start(wh_sb[:], wh_d[:])
            h0_sb = const.tile([128, KH, BL], F32)
            nc.sync.dma_start(h0_sb[:], h0_d.rearrange("p (k j) -> p k j", j=BL))
            if use_bi:
                bi_sb = const.tile([128, M3], F32)
                nc.sync.dma_start(bi_sb[:], bi_d[:])
            if use_bhn:
                bhn_sb = const.tile([128, KH], F32)
                nc.sync.dma_start(bhn_sb[:], bhn_d[:])
            xt_sb = xin.tile([128, KD * TB], BF16)
            nc.sync.dma_start(xt_sb[:], xt_d[:])

            # ---- phase 1: giT = Wi.T @ xT (+ bi), spilled to HBM ----
            for n in range(NT):
                for m in range(M3):
                    ps = p1.tile([128, 512], F32, tag="p1ps")
                    for k in range(KD):
                        nc.tensor.matmul(
                            ps[:],
                            wi_sb[:, (m * KD + k) * 128 : (m * KD + k + 1) * 128],
                            xt_sb[:, k * TB + n * 512 : k * TB + n * 512 + 512],
                            start=(k == 0),
                            stop=(k == KD - 1),
                        )
                    dst = gi_v[:, m, n * (512 // BL) : (n + 1) * (512 // BL), :]
                    st = stagep.tile([128, 512], F32, tag="p1st")
                    if use_bi:
                        nc.vector.tensor_scalar_add(st[:], ps[:], bi_sb[:, m : m + 1])
                    else:
                        nc.vector.tensor_copy(st[:], ps[:])
                    nc.sync.dma_start(dst, st.rearrange("p (t j) -> p t j", j=BL))

            # ---- phase 2: recurrence ----
            # initial bf16 cast of h0
            hb = hbp.tile([128, KH, BL], BF16, tag="hb")
            nc.vector.tensor_copy(hb[:], h0_sb[:])

            h_prev = h0_sb
            o_cur = None
            for t in range(T):
                u = t % RT
                if u == 0:
                    g = girp.tile([128, M3, RT, BL], F32, tag="gir")
                    nc.sync.dma_start(g[:], gi_v[:, :, t : t + RT, :])
                    o_cur = orp.tile([128, KH, RT, BL], F32, tag="oring")

                # gh matmuls: Wh tiles stationary, hT streams (N=BL)
                rz_ps = rzp.tile([128, 8, BL], F32, tag="rzps")
                n_ps = npp.tile([128, KH, BL], F32, tag="nps")
                for m in range(M3):
                    out_ap = rz_ps[:, m, :] if m < 8 else n_ps[:, m - 8, :]
                    for k in range(KH):
                        nc.tensor.matmul(
                            out_ap,
                            wh_sb[:, (m * KH + k) * 128 : (m * KH + k + 1) * 128],
                            hb[:, k, :],
                            start=(k == 0),
                            stop=(k == KH - 1),
                        )

                # elementwise gate math (T-layout tiles)
                pre_rz = ew.tile([128, 8, BL], F32, tag="prerz")
                nc.vector.tensor_add(pre_rz[:], rz_ps[:], g[:, 0:8, u, :])
                rzt = ew.tile([128, 8, BL], F32, tag="rzt")
                nc.scalar.activation(
                    rzt[:], pre_rz[:], mybir.ActivationFunctionType.Sigmoid
                )
                rpn = ew.tile([128, KH, BL], F32, tag="rpn")
                if use_bhn:
                    for k in range(KH):
                        nc.vector.scalar_tensor_tensor(
                            rpn[:, k, :],
                            n_ps[:, k, :],
                            bhn_sb[:, k : k + 1],
                            rzt[:, k, :],
                            mybir.AluOpType.add,
                            mybir.AluOpType.mult,
                        )
                else:
                    nc.vector.tensor_mul(rpn[:], n_ps[:], rzt[:, 0:KH, :])
                pre_n = ew.tile([128, KH, BL], F32, tag="pren")
                nc.vector.tensor_add(pre_n[:], rpn[:], g[:, 8:12, u, :])
                nt = ew.tile([128, KH, BL], F32, tag="nt")
                nc.scalar.activation(
                    nt[:], pre_n[:], mybir.ActivationFunctionType.Tanh
                )
                d = ew.tile([128, KH, BL], F32, tag="d")
                nc.vector.tensor_sub(d[:], h_prev[:], nt[:])
                zd = ew.tile([128, KH, BL], F32, tag="zd")
                nc.vector.tensor_mul(zd[:], rzt[:, KH : 2 * KH, :], d[:])
                h_new = o_cur[:, :, u, :]
                nc.vector.tensor_add(h_new, zd[:], nt[:])
                hb = hbp.tile([128, KH, BL], BF16, tag="hb")
                nc.vector.tensor_copy(hb[:], h_new)
                h_prev = h_new

                if u == RT - 1:
                    nc.sync.dma_start(
                        ys_v[:, :, t - RT + 1 : t + 1, :], o_cur[:]
                    )

    nc.compile()
    return nc


def _get(T, use_bi, use_bhn):
    key = (T, use_bi, use_bhn)
    if key not in _cache:
        _cache[key] = _build(T, use_bi, use_bhn)
    return _cache[key]


def _pack_w(W, kc):
    # W [kc*128, M3*128] -> [128, M3*kc*128], col ((m*kc)+k)*128+c = W[k*128+p, m*128+c]
    return np.ascontiguousarray(
        W.astype(NPBF16).reshape(kc, 128, M3, 128).transpose(1, 2, 0, 3).reshape(128, -1)
    )


def kernel(x, h0, Wi, Wh, bi, bhn, _trace=False, _trace_kwargs=None):
    T = x.shape[0]
    use_bi = bool(np.any(bi))
    use_bhn = bool(np.any(bhn))
    nc = _get(T, use_bi, use_bhn)
    TB = T * BL

    wi_p = _pack_w(np.asarray(Wi), KD)
    wh_p = _pack_w(np.asarray(Wh), KH)
    x = np.asarray(x)
    h0 = np.asarray(h0)

    in_maps = []
    for c in range(NCORES):
        xc = x[:, c * BL : (c + 1) * BL, :]  # [T, BL, D]
        xt = np.ascontiguousarray(
            xc.astype(NPBF16).reshape(T, BL, KD, 128).transpose(3, 2, 0, 1).reshape(128, KD * TB)
        )
        h0c = np.ascontiguousarray(
            h0[c * BL : (c + 1) * BL, :].astype(np.float32).reshape(BL, KH, 128).transpose(2, 1, 0).reshape(128, KH * BL)
        )
        im = {"xt": xt, "wi": wi_p, "wh": wh_p, "h0t": h0c}
        if use_bi:
            im["bi_t"] = np.ascontiguousarray(bi.astype(np.float32).reshape(M3, 128).T)
        if use_bhn:
            im["bhn_t"] = np.ascontiguousarray(bhn.astype(np.float32).reshape(KH, 128).T)
        in_maps.append(im)

    kw = {}
    if _trace:
        kw = dict(trace=True, **(_trace_kwargs or {}))
    kernel._last_in_maps = in_maps
    res = bass_utils.run_bass_kernel_spmd(nc, in_maps, core_ids=list(range(NCORES)), **kw)

    ys = np.empty((T, B, H), dtype=np.float32)
    for c in range(NCORES):
        out = res.results[c]["yst"]  # [128, KH*TB]
        ys[:, c * BL : (c + 1) * BL, :] = (
            out.reshape(128, KH, T, BL).transpose(2, 3, 1, 0).reshape(T, BL, H)
        )
    kernel._last_result = res
    return ys
